# revision 103
# baseline (speedup 1.0000x reference)
"""AFM (Attentional Factorization Machine) Trainium2 kernel, 8-core data parallel.

Reference computation (B=2048, n=64 features, d=64 emb, att=64):
    e[b,i,:]  = x[b,i] * V[i,:]
    prod      = e[:,ii,:] * e[:,jj,:]            (P = 2016 feature pairs)
    h         = relu(prod @ w_att1.T + b_att1)
    score     = h @ w_att2 + b_att2
    att       = softmax(score, axis=pairs)
    pooled    = sum_p att * prod
    out       = sigmoid(pooled @ w_fc + b_fc + x @ w_lin + b_lin)

Algebraic collapse used here (valid because b_att1 == 0 in this problem):
    prod[b,p,:] = S[b,p] * W[p,:]      where S = x[:,ii]*x[:,jj], W = V[ii]*V[jj]
    score[b,p]  = S*cpos[p] if S>=0 else S*cneg[p]
                = S*a[p] + |S|*d[p]
      with A = W @ w_att1.T, cpos = relu(A)@w_att2, cneg = min(A,0)@w_att2,
           a = (cpos+cneg)/2, d = (cpos-cneg)/2
    pooled @ w_fc = (sum_p att*S*u) with u = W @ w_fc
    => out = sigmoid( (sum_p E*S*u)/(sum_p E) + x@w_lin + b_fc + b_lin ),
       E = exp(score)   (scores are O(10), no max-subtraction needed)

The two-sided score is a single parametric relu:
    score = sgn * prelu_alpha(c*S)   with c = |cpos| (guarded), sgn =
    sign(cpos), alpha = cneg/cpos — verified on-silicon that Prelu honours
    per-partition scale/alpha operands (Lrelu ignores alpha, fixed 0.01).

Device layout: pairs p on partitions (16 tiles of 126), local batch (256) on
the free axis.  S is produced by exact bf16 hi/lo one-hot gather matmuls
(Gi/Gj); VectorE evacuates Xi from PSUM (f32) and forms S = Xi*Xj (fp16) and
G = E*S; ScalarE computes the whole score path: sc = prelu(c*S, alpha) in
f32 and E = exp(sgn*sc - 4) in fp16 (the -4 shift keeps E and G in fp16
range; the num/Z softmax ratio is shift-invariant).  Both softmax reductions
run as PE matmuls with the E/G tiles as the stationary operand (moving
operand = the u / ones column).  Measured output max rel err vs the
reference: 7.26e-3 (gate: 2e-2); per-core exec ~23.6us (CoreSim timeline).
"""

import numpy as np

B = 2048
NF = 64          # features
NCORES = 8
BL = B // NCORES  # 256 local batch
NT = 16          # pair tiles
TP = 126         # pairs per tile (partitions)
P = NT * TP      # 2016

_BUILT = {}


def _build_nc(debug=False, compile=True, bufs_work=3, bufs_pmm=2,
              xj_direct=True, a1_act=False, st=2, fp16=True,
              groups=None, chunks=None, direct_mod=2, direct_set=None,
              lrelu=True, warm=0, copy_act_set=(), swdge_x=False,
              swdge_out=False):
    import concourse.mybir as mybir
    from concourse.bacc import Bacc
    from concourse.tile import TileContext
    from contextlib import ExitStack

    F32 = mybir.dt.float32
    BF16 = mybir.dt.bfloat16
    F16 = mybir.dt.float16
    # elementwise compute dtype: fp16 (10-bit mantissa) keeps the final
    # rel err ~1.4e-2 (< the 2e-2 gate, deterministic inputs) and unlocks
    # the DVE 2x/4x modes; exp is shifted by -4 so E=exp(score-4) and
    # G=E*S stay in fp16 range (the num/Z softmax ratio is shift-invariant)
    CD = F16 if fp16 else F32
    AF = mybir.ActivationFunctionType
    OP = mybir.AluOpType

    nc = Bacc()  # Bacc.compile() legalizes multi-wait instructions (the TPB
    # ISA allows one sync wait per instruction) into EventSemaphores
    #
    # bf16 one-hot gathers [128, NT*2*TP]: cols t*252:(t+1)*252 = [Gi_t|Gj_t],
    # rows duplicated (k and 64+k) for the hi/lo x split
    GW = NT * TP
    gpk = nc.declare_dram_parameter("gpk", [2 * NF, 2 * GW], BF16,
                                    isOutput=False)
    # x^T in bf16 hi/lo split stacked along K (rows 0:64 hi, 64:128 lo); one
    # K=128 matmul per gather reconstructs exact-f32 x values in PSUM.
    XW = BL
    xt2 = nc.declare_dram_parameter("xt2", [2 * NF, XW], BF16, isOutput=False)
    # f32 pack:
    #   cols 0:BL       rows 0:65  = x^T plus ones row
    #   col  BL         rows 0:65  = [w_lin; b_fc+b_lin]
    #   cols BL+1:BL+65 rows 0:126 = scal[q, t*4+c] (cneg, cpos-cneg, u, one)
    PK = BL + 1 + NT * 4
    pack = nc.declare_dram_parameter("pack", [128, PK], F32, isOutput=False)
    # u / ones reduction weights in the compute dtype (rhs of the reduction
    # matmuls must match the E/G stationary dtype)
    packh = nc.declare_dram_parameter("packh", [TP, NT, 2], CD, isOutput=False)
    out = nc.declare_dram_parameter("out", [128, 2], F32, isOutput=True)
    if debug:
        dbg_s = nc.declare_dram_parameter("dbg_s", [NT, TP, BL], F32,
                                          isOutput=True)
        dbg_e = nc.declare_dram_parameter("dbg_e", [NT, TP, BL], F32,
                                          isOutput=True)
        dbg_nz = nc.declare_dram_parameter("dbg_nz", [128, 4], F32,
                                           isOutput=True)
        dbg_lin = nc.declare_dram_parameter("dbg_lin", [128, 2], F32,
                                            isOutput=True)

    with TileContext(nc) as tc, ExitStack() as ctx:
        singles = ctx.enter_context(tc.tile_pool(name="singles", bufs=1))
        work = ctx.enter_context(tc.tile_pool(name="work", bufs=bufs_work))
        # DVE-written, PE-read tiles must not share slots (a reuse would add
        # a PE-release wait on a DVE instruction that already waits on ACT)
        gwork = ctx.enter_context(tc.tile_pool(name="gwork", bufs=NT))
        pmm = ctx.enter_context(tc.tile_pool(name="pmm", bufs=bufs_pmm,
                                             space="PSUM"))
        pacc = ctx.enter_context(tc.tile_pool(name="pacc", bufs=1, space="PSUM"))

        # DMA issue order matters: HWDGE issues serialize at ~625ns each, so
        # the tensors the first gather matmul needs (x^T, first gather chunk)
        # go first; the f32/fp16 packs are not needed until ~5us in.
        sb_x2a = singles.tile([2 * NF, XW], BF16)
        sb_x2 = sb_x2a[:, 0:BL]
        if swdge_x:
            nc.gpsimd.dma_start(out=sb_x2a[:, :], in_=xt2[:, :])
        else:
            nc.scalar.dma_start(out=sb_x2a[:, :], in_=xt2[:, :])
        sb_gpk = singles.tile([2 * NF, 2 * GW], BF16)
        # graduated chunks (tile edges) so early gathers start early
        tedges = [0, 2, 6, 10, 16] if chunks is None else chunks
        nc.sync.dma_start(
            out=sb_gpk[:, 2 * tedges[0] * TP:2 * tedges[1] * TP],
            in_=gpk[:, 2 * tedges[0] * TP:2 * tedges[1] * TP])
        sb_pack = singles.tile([128, PK], F32)
        nc.scalar.dma_start(out=sb_pack[:, :], in_=pack[:, :])
        sb_ch = singles.tile([TP, NT, 2], CD)
        nc.scalar.dma_start(out=sb_ch[:, :, :], in_=packh[:, :, :])
        for q in range(1, len(tedges) - 1):
            nc.sync.dma_start(
                out=sb_gpk[:, 2 * tedges[q] * TP:2 * tedges[q + 1] * TP],
                in_=gpk[:, 2 * tedges[q] * TP:2 * tedges[q + 1] * TP])

        def g_cols(t):
            return sb_gpk[:, 2 * t * TP:2 * (t + 1) * TP]
        sb_x = sb_pack[0:NF + 1, 0:BL]
        sb_w = sb_pack[0:NF + 1, BL:BL + 1]
        sb_c = sb_pack[0:TP, BL + 1:PK].rearrange("p (t c) -> p t c", c=4)

        # Per-tile partial reductions: [128, t, c] c: 0,1 = num halves;
        # 2,3 = Z halves.  Each column written exactly once (no PSUM
        # accumulation: start=True zero-marks the whole bank, so interleaved
        # accumulation chains in one bank corrupt each other).
        p_nz = pacc.tile([128, NT, 4], F32)
        p_lin = pacc.tile([128, 2], F32)

        if fp16:
            neg4 = singles.tile([TP, 1], F32)
            nc.vector.memset(neg4[:, :], -4.0)

        if warm:
            # dummy matmuls fill the PE's idle window while the input DMAs
            # stream, so the HAM clock gate is already released (2.4 GHz)
            # when the first real gather arrives (~3.4us of sustained PE
            # activity unthrottles the array)
            wsrc = singles.tile([128, BL], BF16)
            nc.gpsimd.memset(wsrc[:, :], 0.0)
            pwrm = pacc.tile([128, BL], F32)
            for _ in range(warm):
                nc.tensor.matmul(pwrm[:, :], lhsT=wsrc[:, 0:128],
                                 rhs=wsrc[:, :], start=True, stop=True,
                                 skip_group_check=True)

        # variable supertile sizes: small groups at the start (pipeline fills
        # sooner after the first DMA chunk) and at the end (shorter serial
        # drain chain into the final reduction)
        if groups is None:
            if st == 2:
                groups = [1, 1] + [2] * 5 + [1] * 4
            else:
                groups = [st] * (NT // st)
        assert sum(groups) == NT

        tbase = 0
        for s, stg in enumerate(groups):
            ptiles = [tbase + k for k in range(stg)]
            tbase += stg
            # Xi^T, Xj^T gathers: [TP, BL] = Gi2_t^T @ [x_hi; x_lo]^T (exact).
            # st subtiles side by side in one PSUM bank — each column range is
            # written exactly once, so the whole-bank zero-mark of a later
            # start=True does not corrupt earlier data.
            pxi = pmm.tile([TP, stg, BL], F32, tag="pxi")
            pxj = pmm.tile([TP, stg, BL], F32, tag="pxj")
            for k, t in enumerate(ptiles):
                gsl = g_cols(t)
                nc.tensor.matmul(
                    pxi[:, k, :], lhsT=gsl[:, 0:TP],
                    rhs=sb_x2[:, :], start=True, stop=True,
                )
                nc.tensor.matmul(
                    pxj[:, k, :], lhsT=gsl[:, TP:2 * TP],
                    rhs=sb_x2[:, :], start=True, stop=True,
                )

            # Stage Xi through ScalarE (DVE may read at most one PSUM
            # operand).  On alternate supertiles stage Xj through ScalarE
            # too: the product then runs in the DVE fp16 2x mode, balancing
            # the PSUM-evacuation cost between the two engines.
            # on direct groups the S product runs 1c/elem regardless (PSUM
            # operand), so keep xi in f32 there — S gets a single fp16
            # rounding instead of two
            # score entirely on ScalarE:  sc = sgn*prelu_alpha(c*S) with
            # E = exp(sgn*sc' - 4) folding the sign via the exp scale.
            # VectorE evacuates Xi (f32, so S gets a single fp16 rounding)
            # and computes S and G.
            xi_sb = work.tile([TP, stg, BL], F32, tag="xi")
            nc.vector.tensor_copy(out=xi_sb[:, :, :], in_=pxi[:, :, :])
            s_t = work.tile([TP, stg, BL], CD, tag="s")
            nc.vector.tensor_tensor(out=s_t[:, :, :], in0=xi_sb[:, :, :],
                                    in1=pxj[:, :, :], op=OP.mult)
            sc = work.tile([TP, stg, BL], F32, tag="sc")
            for k, t in enumerate(ptiles):
                nc.scalar.activation(
                    out=sc[:, k, :], in_=s_t[:, k, :], func=AF.Prelu,
                    scale=sb_c[:, t, 0:1], alpha=sb_c[:, t, 1:2])
            e_t = work.tile([TP, stg, BL], CD, tag="e")
            for k, t in enumerate(ptiles):
                nc.scalar.activation(
                    out=e_t[:, k, :], in_=sc[:, k, :], func=AF.Exp,
                    scale=sb_c[:, t, 2:3], bias=neg4[:, 0:1])
            g_t = gwork.tile([TP, stg, BL], CD, tag="g")
            nc.vector.tensor_tensor(out=g_t[:, :, :], in0=e_t[:, :, :],
                                    in1=s_t[:, :, :], op=OP.mult)
            if debug:
                for k, t in enumerate(ptiles):
                    nc.sync.dma_start(out=dbg_s[t, :, :], in_=s_t[:, k, :])
                    nc.sync.dma_start(out=dbg_e[t, :, :], in_=e_t[:, k, :])

            for k, t in enumerate(ptiles):
                for h in range(2):
                    bsl = slice(h * 128, (h + 1) * 128)
                    nc.tensor.matmul(
                        p_nz[:, t, h:h + 1], lhsT=g_t[:, k, bsl],
                        rhs=sb_ch[:, t, 0:1],
                        start=True, stop=True, skip_group_check=True,
                    )
                    nc.tensor.matmul(
                        p_nz[:, t, 2 + h:3 + h], lhsT=e_t[:, k, bsl],
                        rhs=sb_ch[:, t, 1:2],
                        start=True, stop=True, skip_group_check=True,
                    )

        # x @ w_lin (+ bias via the ones row): one matmul per b-half
        for h in range(2):
            nc.tensor.matmul(
                p_lin[:, h:h + 1],
                lhsT=sb_x[:, h * 128:(h + 1) * 128],
                rhs=sb_w[:, :],
                start=True, stop=True,
            )

        # sum the 16 per-tile partials: [128, (c,t)] -> [128, 4]
        nz = work.tile([128, 4], F32, tag="fin4")
        nc.vector.reduce_sum(
            out=nz[:, :], in_=p_nz[:, :, :].rearrange("p t c -> p c t"),
            axis=mybir.AxisListType.X,
        )

        if debug:
            nc.sync.dma_start(out=dbg_nz[:, :], in_=nz[:, :])
            clin = work.tile([128, 2], F32, tag="dbgc2")
            nc.scalar.activation(out=clin[:, :], in_=p_lin[:, :], func=AF.Copy)
            nc.sync.dma_start(out=dbg_lin[:, :], in_=clin[:, :])

        # logits = num/Z + xlin ; out = 1/(1+exp(-logits))
        rz = work.tile([128, 2], F32, tag="fin")
        nc.vector.reciprocal(rz[:, :], nz[:, 2:4])
        # logit col h = num_h * (1/Z_h) + lin_h, fused per column
        lg = work.tile([128, 2], F32, tag="fin")
        for h in range(2):
            nc.vector.scalar_tensor_tensor(
                out=lg[:, h:h + 1], in0=nz[:, h:h + 1],
                scalar=rz[:, h:h + 1], in1=p_lin[:, h:h + 1],
                op0=OP.mult, op1=OP.add)
        # sigmoid(x) = (1 + tanh(x/2)) / 2 — tanh shares the ACT table set
        # with exp (no table switch), and the affine is one dual-op TS
        th = work.tile([128, 2], F32, tag="fin")
        nc.scalar.activation(out=th[:, :], in_=lg[:, :], func=AF.Tanh,
                             scale=0.5)
        o = work.tile([128, 2], F32, tag="fin")
        nc.vector.tensor_scalar(out=o[:, :], in0=th[:, :], scalar1=1.0,
                                scalar2=0.5, op0=OP.add, op1=OP.mult)
        if swdge_out:
            nc.gpsimd.dma_start(out=out[:, :], in_=o[:, :])
        else:
            nc.sync.dma_start(out=out[:, :], in_=o[:, :])

    if compile:
        nc.compile()
    return nc


def _get_nc():
    if "nc" not in _BUILT:
        _BUILT["nc"] = _build_nc(bufs_work=10, direct_set=(0, 2, 4, 6))
    return _BUILT["nc"]


def _host_prep(x, V, w_att1, b_att1, w_att2, b_att2, w_fc, b_fc, w_lin, b_lin):
    """Fold the tiny replicated parameters into per-pair vectors (float64)."""
    ii, jj = np.triu_indices(NF, k=1)
    V64 = V.astype(np.float64)
    W = V64[ii] * V64[jj]                                  # [P, d]
    A = W @ w_att1.astype(np.float64).T                    # [P, att]
    w2 = w_att2.astype(np.float64)
    cpos = np.maximum(A, 0.0) @ w2
    cneg = np.minimum(A, 0.0) @ w2
    avec = 0.5 * (cpos + cneg)
    dvec = 0.5 * (cpos - cneg)
    u = W @ w_fc.astype(np.float64)
    const = float(b_fc) + float(b_lin)

    import ml_dtypes

    pidx = np.arange(P)
    t_, m_ = pidx // TP, pidx % TP

    # bf16 one-hot gathers, [Gi_t | Gj_t] interleaved per tile, rows
    # duplicated for the hi/lo x split
    GW = NT * TP
    gpk = np.zeros((2 * NF, 2 * GW), ml_dtypes.bfloat16)
    gpk[ii, 2 * t_ * TP + m_] = 1.0
    gpk[NF + ii, 2 * t_ * TP + m_] = 1.0
    gpk[jj, (2 * t_ + 1) * TP + m_] = 1.0
    gpk[NF + jj, (2 * t_ + 1) * TP + m_] = 1.0

    # score = cpos*S (S>=0) | cneg*S (S<0)  ==  sgn * lrelu_alpha(c*S)
    # with c = |cpos| (guarded), sgn = sign(cpos), alpha = cneg/(sgn*c)
    sgn = np.where(cpos >= 0, 1.0, -1.0)
    c_safe = np.maximum(np.abs(cpos), 1e-20)
    alpha = cneg / (sgn * c_safe)
    scal = np.zeros((TP, NT, 4), np.float32)
    scal[m_, t_, 0] = c_safe
    scal[m_, t_, 1] = alpha
    scal[m_, t_, 2] = sgn
    scal[m_, t_, 3] = cneg                # legacy path (lrelu=False)
    scal2 = np.zeros((TP, NT, 2), np.float32)
    scal2[m_, t_, 0] = cpos - cneg        # legacy path
    scal2[m_, t_, 1] = 1.0

    wla = np.concatenate([w_lin.astype(np.float64), [const]]).astype(
        np.float32).reshape(NF + 1, 1)

    # packed f32 blob minus the per-core xTa block (filled in kernel())
    PK = BL + 1 + NT * 4
    pack = np.zeros((128, PK), np.float32)
    pack[0:NF + 1, BL:BL + 1] = wla
    pack[0:TP, BL + 1:PK] = scal.reshape(TP, NT * 4)
    # u / ones reduction weights in fp16 for the fp16 pipeline
    packh = np.zeros((TP, NT, 2), np.float16)
    packh[m_, t_, 0] = u
    packh[m_, t_, 1] = 1.0
    return gpk, pack, packh


def kernel(**inputs):
    from concourse.bass_utils import run_bass_kernel_spmd

    x = np.ascontiguousarray(np.asarray(inputs["x"], dtype=np.float32))
    assert x.shape == (B, NF), x.shape
    b_att1 = np.asarray(inputs["b_att1"], dtype=np.float64)
    assert np.allclose(b_att1, 0.0), "kernel specialization requires b_att1 == 0"

    gpk0, pack0, packh0 = _host_prep(
        x, np.asarray(inputs["V"]), np.asarray(inputs["w_att1"]), b_att1,
        np.asarray(inputs["w_att2"]), np.asarray(inputs["b_att2"]),
        np.asarray(inputs["w_fc"]), np.asarray(inputs["b_fc"]),
        np.asarray(inputs["w_lin"]), np.asarray(inputs["b_lin"]),
    )

    import ml_dtypes

    x_hi32 = x.astype(ml_dtypes.bfloat16).astype(np.float32)
    x_lo = (x - x_hi32).astype(ml_dtypes.bfloat16)
    x_hi = x_hi32.astype(ml_dtypes.bfloat16)

    in_maps = []
    for c in range(NCORES):
        sl = slice(c * BL, (c + 1) * BL)
        pack = pack0.copy()
        pack[0:NF, 0:BL] = x[sl].T
        pack[NF, 0:BL] = 1.0
        xt2 = np.concatenate([x_hi[sl].T, x_lo[sl].T], axis=0)  # [128, BL]
        in_maps.append({"gpk": gpk0, "xt2": np.ascontiguousarray(xt2),
                        "pack": pack, "packh": packh0})

    nc = _get_nc()
    results = run_bass_kernel_spmd(nc, in_maps, core_ids=list(range(NCORES)))

    outs = []
    for c in range(NCORES):
        res = results.results[c]["out"]                   # [128, 2]
        outs.append(res.T.reshape(-1))                    # b_local = h*128 + q
    return np.concatenate(outs).astype(np.float32)


# revision 108
# speedup vs baseline: 1.0068x; 1.0068x over previous
"""AFM (Attentional Factorization Machine) Trainium2 kernel, 8-core data parallel.

Reference computation (B=2048, n=64 features, d=64 emb, att=64):
    e[b,i,:]  = x[b,i] * V[i,:]
    prod      = e[:,ii,:] * e[:,jj,:]            (P = 2016 feature pairs)
    h         = relu(prod @ w_att1.T + b_att1)
    score     = h @ w_att2 + b_att2
    att       = softmax(score, axis=pairs)
    pooled    = sum_p att * prod
    out       = sigmoid(pooled @ w_fc + b_fc + x @ w_lin + b_lin)

Algebraic collapse used here (valid because b_att1 == 0 in this problem):
    prod[b,p,:] = S[b,p] * W[p,:]      where S = x[:,ii]*x[:,jj], W = V[ii]*V[jj]
    score[b,p]  = S*cpos[p] if S>=0 else S*cneg[p]
                = S*a[p] + |S|*d[p]
      with A = W @ w_att1.T, cpos = relu(A)@w_att2, cneg = min(A,0)@w_att2,
           a = (cpos+cneg)/2, d = (cpos-cneg)/2
    pooled @ w_fc = (sum_p att*S*u) with u = W @ w_fc
    => out = sigmoid( (sum_p E*S*u)/(sum_p E) + x@w_lin + b_fc + b_lin ),
       E = exp(score)   (scores are O(10), no max-subtraction needed)

The two-sided score is a single parametric relu:
    score = sgn * prelu_alpha(c*S)   with c = |cpos| (guarded), sgn =
    sign(cpos), alpha = cneg/cpos — verified on-silicon that Prelu honours
    per-partition scale/alpha operands (Lrelu ignores alpha, fixed 0.01).

Device layout: pairs p on partitions (16 tiles of 126), local batch (256) on
the free axis.  S is produced by exact bf16 hi/lo one-hot gather matmuls
(Gi/Gj); VectorE evacuates Xi from PSUM (f32) and forms S = Xi*Xj (fp16) and
G = E*S; ScalarE computes the whole score path: sc = prelu(c*S, alpha) in
f32 and E = exp(sgn*sc - 4) in fp16 (the -4 shift keeps E and G in fp16
range; the num/Z softmax ratio is shift-invariant).  Both softmax reductions
run as PE matmuls with the E/G tiles as the stationary operand (moving
operand = the u / ones column).  Measured output max rel err vs the
reference: 7.26e-3 (gate: 2e-2); per-core exec ~23.6us (CoreSim timeline).
"""

import numpy as np

B = 2048
NF = 64          # features
NCORES = 8
BL = B // NCORES  # 256 local batch
NT = 16          # pair tiles
TP = 126         # pairs per tile (partitions)
P = NT * TP      # 2016

_BUILT = {}
DVE_SCORE = 1   # trailing singleton group computes score on VectorE


def _build_nc(debug=False, compile=True, bufs_work=3, bufs_pmm=2,
              xj_direct=True, a1_act=False, st=2, fp16=True,
              groups=None, chunks=None, direct_mod=2, direct_set=None,
              lrelu=True, warm=0, copy_act_set=(), swdge_x=False,
              swdge_out=False, pool_mode="stack", dve_score=0):
    import concourse.mybir as mybir
    from concourse.bacc import Bacc
    from concourse.tile import TileContext
    from contextlib import ExitStack

    F32 = mybir.dt.float32
    BF16 = mybir.dt.bfloat16
    F16 = mybir.dt.float16
    # elementwise compute dtype: fp16 (10-bit mantissa) keeps the final
    # rel err ~1.4e-2 (< the 2e-2 gate, deterministic inputs) and unlocks
    # the DVE 2x/4x modes; exp is shifted by -4 so E=exp(score-4) and
    # G=E*S stay in fp16 range (the num/Z softmax ratio is shift-invariant)
    CD = F16 if fp16 else F32
    AF = mybir.ActivationFunctionType
    OP = mybir.AluOpType

    nc = Bacc()  # Bacc.compile() legalizes multi-wait instructions (the TPB
    # ISA allows one sync wait per instruction) into EventSemaphores
    #
    # bf16 one-hot gathers [128, NT*2*TP]: cols t*252:(t+1)*252 = [Gi_t|Gj_t],
    # rows duplicated (k and 64+k) for the hi/lo x split
    GW = NT * TP
    gpk = nc.declare_dram_parameter("gpk", [2 * NF, 2 * GW], BF16,
                                    isOutput=False)
    # x^T in bf16 hi/lo split stacked along K (rows 0:64 hi, 64:128 lo); one
    # K=128 matmul per gather reconstructs exact-f32 x values in PSUM.
    XW = BL
    xt2 = nc.declare_dram_parameter("xt2", [2 * NF, XW], BF16, isOutput=False)
    # f32 pack:
    #   cols 0:BL       rows 0:65  = x^T plus ones row
    #   col  BL         rows 0:65  = [w_lin; b_fc+b_lin]
    #   cols BL+1:BL+65 rows 0:126 = scal[q, t*4+c] (cneg, cpos-cneg, u, one)
    PK = BL + 1 + NT * 4
    pack = nc.declare_dram_parameter("pack", [128, PK], F32, isOutput=False)
    # u / ones reduction weights in the compute dtype (rhs of the reduction
    # matmuls must match the E/G stationary dtype)
    packh = nc.declare_dram_parameter("packh", [TP, NT, 2], CD, isOutput=False)
    out = nc.declare_dram_parameter("out", [128, 2], F32, isOutput=True)
    if debug:
        dbg_s = nc.declare_dram_parameter("dbg_s", [NT, TP, BL], F32,
                                          isOutput=True)
        dbg_e = nc.declare_dram_parameter("dbg_e", [NT, TP, BL], F32,
                                          isOutput=True)
        dbg_nz = nc.declare_dram_parameter("dbg_nz", [128, 4], F32,
                                           isOutput=True)
        dbg_lin = nc.declare_dram_parameter("dbg_lin", [128, 2], F32,
                                            isOutput=True)

    with TileContext(nc, pool_alloc_mode=pool_mode) as tc, \
            ExitStack() as ctx:
        singles = ctx.enter_context(tc.tile_pool(name="singles", bufs=1))
        work = ctx.enter_context(tc.tile_pool(name="work", bufs=bufs_work))
        # DVE-written, PE-read tiles must not share slots (a reuse would add
        # a PE-release wait on a DVE instruction that already waits on ACT)
        gwork = ctx.enter_context(tc.tile_pool(name="gwork", bufs=NT))
        pmm = ctx.enter_context(tc.tile_pool(name="pmm", bufs=bufs_pmm,
                                             space="PSUM"))
        pacc = ctx.enter_context(tc.tile_pool(name="pacc", bufs=1, space="PSUM"))

        # DMA issue order matters: HWDGE issues serialize at ~625ns each, so
        # the tensors the first gather matmul needs (x^T, first gather chunk)
        # go first; the f32/fp16 packs are not needed until ~5us in.
        sb_x2a = singles.tile([2 * NF, XW], BF16)
        sb_x2 = sb_x2a[:, 0:BL]
        if swdge_x:
            nc.gpsimd.dma_start(out=sb_x2a[:, :], in_=xt2[:, :])
        else:
            nc.scalar.dma_start(out=sb_x2a[:, :], in_=xt2[:, :])
        sb_gpk = singles.tile([2 * NF, 2 * GW], BF16)
        # graduated chunks (tile edges) so early gathers start early
        tedges = [0, 2, 6, 10, 16] if chunks is None else chunks
        nc.sync.dma_start(
            out=sb_gpk[:, 2 * tedges[0] * TP:2 * tedges[1] * TP],
            in_=gpk[:, 2 * tedges[0] * TP:2 * tedges[1] * TP])
        sb_pack = singles.tile([128, PK], F32)
        nc.scalar.dma_start(out=sb_pack[:, :], in_=pack[:, :])
        sb_ch = singles.tile([TP, NT, 2], CD)
        nc.scalar.dma_start(out=sb_ch[:, :, :], in_=packh[:, :, :])
        for q in range(1, len(tedges) - 1):
            nc.sync.dma_start(
                out=sb_gpk[:, 2 * tedges[q] * TP:2 * tedges[q + 1] * TP],
                in_=gpk[:, 2 * tedges[q] * TP:2 * tedges[q + 1] * TP])

        def g_cols(t):
            return sb_gpk[:, 2 * t * TP:2 * (t + 1) * TP]
        sb_x = sb_pack[0:NF + 1, 0:BL]
        sb_w = sb_pack[0:NF + 1, BL:BL + 1]
        sb_c = sb_pack[0:TP, BL + 1:PK].rearrange("p (t c) -> p t c", c=4)

        # Per-tile partial reductions: [128, t, c] c: 0,1 = num halves;
        # 2,3 = Z halves.  Each column written exactly once (no PSUM
        # accumulation: start=True zero-marks the whole bank, so interleaved
        # accumulation chains in one bank corrupt each other).
        p_nz = pacc.tile([128, NT, 4], F32)
        p_lin = pacc.tile([128, 2], F32)

        if fp16:
            neg4 = singles.tile([TP, 1], F32)
            nc.vector.memset(neg4[:, :], -4.0)

        if warm:
            # dummy matmuls fill the PE's idle window while the input DMAs
            # stream, so the HAM clock gate is already released (2.4 GHz)
            # when the first real gather arrives (~3.4us of sustained PE
            # activity unthrottles the array)
            wsrc = singles.tile([128, BL], BF16)
            nc.gpsimd.memset(wsrc[:, :], 0.0)
            pwrm = pacc.tile([128, BL], F32)
            for _ in range(warm):
                nc.tensor.matmul(pwrm[:, :], lhsT=wsrc[:, 0:128],
                                 rhs=wsrc[:, :], start=True, stop=True,
                                 skip_group_check=True)

        # variable supertile sizes: small groups at the start (pipeline fills
        # sooner after the first DMA chunk) and at the end (shorter serial
        # drain chain into the final reduction)
        if groups is None:
            if st == 2:
                groups = [1, 1] + [2] * 5 + [1] * 4
            else:
                groups = [st] * (NT // st)
        assert sum(groups) == NT

        tbase = 0
        for s, stg in enumerate(groups):
            ptiles = [tbase + k for k in range(stg)]
            tbase += stg
            # Xi^T, Xj^T gathers: [TP, BL] = Gi2_t^T @ [x_hi; x_lo]^T (exact).
            # st subtiles side by side in one PSUM bank — each column range is
            # written exactly once, so the whole-bank zero-mark of a later
            # start=True does not corrupt earlier data.
            pxi = pmm.tile([TP, stg, BL], F32, tag="pxi")
            pxj = pmm.tile([TP, stg, BL], F32, tag="pxj")
            for k, t in enumerate(ptiles):
                gsl = g_cols(t)
                nc.tensor.matmul(
                    pxi[:, k, :], lhsT=gsl[:, 0:TP],
                    rhs=sb_x2[:, :], start=True, stop=True,
                )
                nc.tensor.matmul(
                    pxj[:, k, :], lhsT=gsl[:, TP:2 * TP],
                    rhs=sb_x2[:, :], start=True, stop=True,
                )

            # Stage Xi through ScalarE (DVE may read at most one PSUM
            # operand).  On alternate supertiles stage Xj through ScalarE
            # too: the product then runs in the DVE fp16 2x mode, balancing
            # the PSUM-evacuation cost between the two engines.
            # on direct groups the S product runs 1c/elem regardless (PSUM
            # operand), so keep xi in f32 there — S gets a single fp16
            # rounding instead of two
            # score entirely on ScalarE:  sc = sgn*prelu_alpha(c*S) with
            # E = exp(sgn*sc' - 4) folding the sign via the exp scale.
            # VectorE evacuates Xi (f32, so S gets a single fp16 rounding)
            # and computes S and G.
            xi_sb = work.tile([TP, stg, BL], F32, tag="xi")
            nc.vector.tensor_copy(out=xi_sb[:, :, :], in_=pxi[:, :, :])
            s_t = work.tile([TP, stg, BL], CD, tag="s")
            nc.vector.tensor_tensor(out=s_t[:, :, :], in0=xi_sb[:, :, :],
                                    in1=pxj[:, :, :], op=OP.mult)
            sc = work.tile([TP, stg, BL], F32, tag="sc")
            e_t = work.tile([TP, stg, BL], CD, tag="e")
            if s >= len(groups) - dve_score:
                # tail groups: score on VectorE (ACT is the serial driver
                # in the drain); those tiles' scal cols hold (cpmn, cneg)
                a1 = work.tile([TP, stg, BL], CD, tag="a1")
                for k, t in enumerate(ptiles):
                    nc.vector.tensor_scalar(
                        out=a1[:, k, :], in0=s_t[:, k, :], scalar1=0.0,
                        scalar2=sb_c[:, t, 0:1], op0=OP.max, op1=OP.mult)
                for k, t in enumerate(ptiles):
                    nc.vector.scalar_tensor_tensor(
                        out=sc[:, k, :], in0=s_t[:, k, :],
                        scalar=sb_c[:, t, 1:2], in1=a1[:, k, :],
                        op0=OP.mult, op1=OP.add)
                nc.scalar.activation(out=e_t[:, :, :], in_=sc[:, :, :],
                                     func=AF.Exp, bias=neg4[:, 0:1])
            else:
                for k, t in enumerate(ptiles):
                    nc.scalar.activation(
                        out=sc[:, k, :], in_=s_t[:, k, :], func=AF.Prelu,
                        scale=sb_c[:, t, 0:1], alpha=sb_c[:, t, 1:2])
                for k, t in enumerate(ptiles):
                    nc.scalar.activation(
                        out=e_t[:, k, :], in_=sc[:, k, :], func=AF.Exp,
                        scale=sb_c[:, t, 2:3], bias=neg4[:, 0:1])
            g_t = gwork.tile([TP, stg, BL], CD, tag="g")
            nc.vector.tensor_tensor(out=g_t[:, :, :], in0=e_t[:, :, :],
                                    in1=s_t[:, :, :], op=OP.mult)
            if debug:
                for k, t in enumerate(ptiles):
                    nc.sync.dma_start(out=dbg_s[t, :, :], in_=s_t[:, k, :])
                    nc.sync.dma_start(out=dbg_e[t, :, :], in_=e_t[:, k, :])

            for k, t in enumerate(ptiles):
                for h in range(2):
                    bsl = slice(h * 128, (h + 1) * 128)
                    nc.tensor.matmul(
                        p_nz[:, t, h:h + 1], lhsT=g_t[:, k, bsl],
                        rhs=sb_ch[:, t, 0:1],
                        start=True, stop=True, skip_group_check=True,
                    )
                    nc.tensor.matmul(
                        p_nz[:, t, 2 + h:3 + h], lhsT=e_t[:, k, bsl],
                        rhs=sb_ch[:, t, 1:2],
                        start=True, stop=True, skip_group_check=True,
                    )

        # x @ w_lin (+ bias via the ones row): one matmul per b-half
        for h in range(2):
            nc.tensor.matmul(
                p_lin[:, h:h + 1],
                lhsT=sb_x[:, h * 128:(h + 1) * 128],
                rhs=sb_w[:, :],
                start=True, stop=True,
            )

        # sum the 16 per-tile partials: [128, (c,t)] -> [128, 4]
        nz = work.tile([128, 4], F32, tag="fin4")
        nc.vector.reduce_sum(
            out=nz[:, :], in_=p_nz[:, :, :].rearrange("p t c -> p c t"),
            axis=mybir.AxisListType.X,
        )

        if debug:
            nc.sync.dma_start(out=dbg_nz[:, :], in_=nz[:, :])
            clin = work.tile([128, 2], F32, tag="dbgc2")
            nc.scalar.activation(out=clin[:, :], in_=p_lin[:, :], func=AF.Copy)
            nc.sync.dma_start(out=dbg_lin[:, :], in_=clin[:, :])

        # logits = num/Z + xlin ; out = 1/(1+exp(-logits))
        rz = work.tile([128, 2], F32, tag="fin")
        nc.vector.reciprocal(rz[:, :], nz[:, 2:4])
        # logit col h = num_h * (1/Z_h) + lin_h, fused per column
        lg = work.tile([128, 2], F32, tag="fin")
        for h in range(2):
            nc.vector.scalar_tensor_tensor(
                out=lg[:, h:h + 1], in0=nz[:, h:h + 1],
                scalar=rz[:, h:h + 1], in1=p_lin[:, h:h + 1],
                op0=OP.mult, op1=OP.add)
        # sigmoid(x) = (1 + tanh(x/2)) / 2 — tanh shares the ACT table set
        # with exp (no table switch), and the affine is one dual-op TS
        th = work.tile([128, 2], F32, tag="fin")
        nc.scalar.activation(out=th[:, :], in_=lg[:, :], func=AF.Tanh,
                             scale=0.5)
        o = work.tile([128, 2], F32, tag="fin")
        nc.vector.tensor_scalar(out=o[:, :], in0=th[:, :], scalar1=1.0,
                                scalar2=0.5, op0=OP.add, op1=OP.mult)
        if swdge_out:
            nc.gpsimd.dma_start(out=out[:, :], in_=o[:, :])
        else:
            nc.sync.dma_start(out=out[:, :], in_=o[:, :])

    if compile:
        nc.compile()
    return nc


def _get_nc():
    if "nc" not in _BUILT:
        _BUILT["nc"] = _build_nc(bufs_work=10, dve_score=DVE_SCORE)
    return _BUILT["nc"]


def _host_prep(x, V, w_att1, b_att1, w_att2, b_att2, w_fc, b_fc, w_lin, b_lin):
    """Fold the tiny replicated parameters into per-pair vectors (float64)."""
    ii, jj = np.triu_indices(NF, k=1)
    V64 = V.astype(np.float64)
    W = V64[ii] * V64[jj]                                  # [P, d]
    A = W @ w_att1.astype(np.float64).T                    # [P, att]
    w2 = w_att2.astype(np.float64)
    cpos = np.maximum(A, 0.0) @ w2
    cneg = np.minimum(A, 0.0) @ w2
    avec = 0.5 * (cpos + cneg)
    dvec = 0.5 * (cpos - cneg)
    u = W @ w_fc.astype(np.float64)
    const = float(b_fc) + float(b_lin)

    import ml_dtypes

    pidx = np.arange(P)
    t_, m_ = pidx // TP, pidx % TP

    # bf16 one-hot gathers, [Gi_t | Gj_t] interleaved per tile, rows
    # duplicated for the hi/lo x split
    GW = NT * TP
    gpk = np.zeros((2 * NF, 2 * GW), ml_dtypes.bfloat16)
    gpk[ii, 2 * t_ * TP + m_] = 1.0
    gpk[NF + ii, 2 * t_ * TP + m_] = 1.0
    gpk[jj, (2 * t_ + 1) * TP + m_] = 1.0
    gpk[NF + jj, (2 * t_ + 1) * TP + m_] = 1.0

    # score = cpos*S (S>=0) | cneg*S (S<0)  ==  sgn * lrelu_alpha(c*S)
    # with c = |cpos| (guarded), sgn = sign(cpos), alpha = cneg/(sgn*c)
    sgn = np.where(cpos >= 0, 1.0, -1.0)
    c_safe = np.maximum(np.abs(cpos), 1e-20)
    alpha = cneg / (sgn * c_safe)
    scal = np.zeros((TP, NT, 4), np.float32)
    scal[m_, t_, 0] = c_safe
    scal[m_, t_, 1] = alpha
    scal[m_, t_, 2] = sgn
    scal[m_, t_, 3] = cneg                # legacy path (lrelu=False)
    # trailing tiles use the VectorE score path: cols hold (cpmn, cneg)
    sel = t_ >= NT - DVE_SCORE
    scal[m_[sel], t_[sel], 0] = (cpos - cneg)[sel]
    scal[m_[sel], t_[sel], 1] = cneg[sel]
    scal2 = np.zeros((TP, NT, 2), np.float32)
    scal2[m_, t_, 0] = cpos - cneg        # legacy path
    scal2[m_, t_, 1] = 1.0

    wla = np.concatenate([w_lin.astype(np.float64), [const]]).astype(
        np.float32).reshape(NF + 1, 1)

    # packed f32 blob minus the per-core xTa block (filled in kernel())
    PK = BL + 1 + NT * 4
    pack = np.zeros((128, PK), np.float32)
    pack[0:NF + 1, BL:BL + 1] = wla
    pack[0:TP, BL + 1:PK] = scal.reshape(TP, NT * 4)
    # u / ones reduction weights in fp16 for the fp16 pipeline
    packh = np.zeros((TP, NT, 2), np.float16)
    packh[m_, t_, 0] = u
    packh[m_, t_, 1] = 1.0
    return gpk, pack, packh


def kernel(**inputs):
    from concourse.bass_utils import run_bass_kernel_spmd

    x = np.ascontiguousarray(np.asarray(inputs["x"], dtype=np.float32))
    assert x.shape == (B, NF), x.shape
    b_att1 = np.asarray(inputs["b_att1"], dtype=np.float64)
    assert np.allclose(b_att1, 0.0), "kernel specialization requires b_att1 == 0"

    gpk0, pack0, packh0 = _host_prep(
        x, np.asarray(inputs["V"]), np.asarray(inputs["w_att1"]), b_att1,
        np.asarray(inputs["w_att2"]), np.asarray(inputs["b_att2"]),
        np.asarray(inputs["w_fc"]), np.asarray(inputs["b_fc"]),
        np.asarray(inputs["w_lin"]), np.asarray(inputs["b_lin"]),
    )

    import ml_dtypes

    x_hi32 = x.astype(ml_dtypes.bfloat16).astype(np.float32)
    x_lo = (x - x_hi32).astype(ml_dtypes.bfloat16)
    x_hi = x_hi32.astype(ml_dtypes.bfloat16)

    in_maps = []
    for c in range(NCORES):
        sl = slice(c * BL, (c + 1) * BL)
        pack = pack0.copy()
        pack[0:NF, 0:BL] = x[sl].T
        pack[NF, 0:BL] = 1.0
        xt2 = np.concatenate([x_hi[sl].T, x_lo[sl].T], axis=0)  # [128, BL]
        in_maps.append({"gpk": gpk0, "xt2": np.ascontiguousarray(xt2),
                        "pack": pack, "packh": packh0})

    nc = _get_nc()
    results = run_bass_kernel_spmd(nc, in_maps, core_ids=list(range(NCORES)))

    outs = []
    for c in range(NCORES):
        res = results.results[c]["out"]                   # [128, 2]
        outs.append(res.T.reshape(-1))                    # b_local = h*128 + q
    return np.concatenate(outs).astype(np.float32)


# revision 109
# speedup vs baseline: 1.0076x; 1.0008x over previous
"""AFM (Attentional Factorization Machine) Trainium2 kernel, 8-core data parallel.

Reference computation (B=2048, n=64 features, d=64 emb, att=64):
    e[b,i,:]  = x[b,i] * V[i,:]
    prod      = e[:,ii,:] * e[:,jj,:]            (P = 2016 feature pairs)
    h         = relu(prod @ w_att1.T + b_att1)
    score     = h @ w_att2 + b_att2
    att       = softmax(score, axis=pairs)
    pooled    = sum_p att * prod
    out       = sigmoid(pooled @ w_fc + b_fc + x @ w_lin + b_lin)

Algebraic collapse used here (valid because b_att1 == 0 in this problem):
    prod[b,p,:] = S[b,p] * W[p,:]      where S = x[:,ii]*x[:,jj], W = V[ii]*V[jj]
    score[b,p]  = S*cpos[p] if S>=0 else S*cneg[p]
                = S*a[p] + |S|*d[p]
      with A = W @ w_att1.T, cpos = relu(A)@w_att2, cneg = min(A,0)@w_att2,
           a = (cpos+cneg)/2, d = (cpos-cneg)/2
    pooled @ w_fc = (sum_p att*S*u) with u = W @ w_fc
    => out = sigmoid( (sum_p E*S*u)/(sum_p E) + x@w_lin + b_fc + b_lin ),
       E = exp(score)   (scores are O(10), no max-subtraction needed)

The two-sided score is a single parametric relu:
    score = sgn * prelu_alpha(c*S)   with c = |cpos| (guarded), sgn =
    sign(cpos), alpha = cneg/cpos — verified on-silicon that Prelu honours
    per-partition scale/alpha operands (Lrelu ignores alpha, fixed 0.01).

Device layout: pairs p on partitions (16 tiles of 126), local batch (256) on
the free axis.  S is produced by exact bf16 hi/lo one-hot gather matmuls
(Gi/Gj); VectorE evacuates Xi from PSUM (f32) and forms S = Xi*Xj (fp16) and
G = E*S; ScalarE computes the whole score path: sc = prelu(c*S, alpha) in
f32 and E = exp(sgn*sc - 4) in fp16 (the -4 shift keeps E and G in fp16
range; the num/Z softmax ratio is shift-invariant).  Both softmax reductions
run as PE matmuls with the E/G tiles as the stationary operand (moving
operand = the u / ones column).  Measured output max rel err vs the
reference: 7.26e-3 (gate: 2e-2); per-core exec ~23.6us (CoreSim timeline).
"""

import numpy as np

B = 2048
NF = 64          # features
NCORES = 8
BL = B // NCORES  # 256 local batch
NT = 16          # pair tiles
TP = 126         # pairs per tile (partitions)
P = NT * TP      # 2016

_BUILT = {}
DVE_SCORE = 1   # trailing singleton group computes score on VectorE


def _build_nc(debug=False, compile=True, bufs_work=3, bufs_pmm=2,
              xj_direct=True, a1_act=False, st=2, fp16=True,
              groups=None, chunks=None, direct_mod=2, direct_set=None,
              lrelu=True, warm=0, copy_act_set=(), swdge_x=False,
              swdge_out=False, pool_mode="stack", dve_score=0):
    import concourse.mybir as mybir
    from concourse.bacc import Bacc
    from concourse.tile import TileContext
    from contextlib import ExitStack

    F32 = mybir.dt.float32
    BF16 = mybir.dt.bfloat16
    F16 = mybir.dt.float16
    # elementwise compute dtype: fp16 (10-bit mantissa) keeps the final
    # rel err ~1.4e-2 (< the 2e-2 gate, deterministic inputs) and unlocks
    # the DVE 2x/4x modes; exp is shifted by -4 so E=exp(score-4) and
    # G=E*S stay in fp16 range (the num/Z softmax ratio is shift-invariant)
    CD = F16 if fp16 else F32
    AF = mybir.ActivationFunctionType
    OP = mybir.AluOpType

    nc = Bacc()  # Bacc.compile() legalizes multi-wait instructions (the TPB
    # ISA allows one sync wait per instruction) into EventSemaphores
    #
    # bf16 one-hot gathers [128, NT*2*TP]: cols t*252:(t+1)*252 = [Gi_t|Gj_t],
    # rows duplicated (k and 64+k) for the hi/lo x split
    GW = NT * TP
    gpk = nc.declare_dram_parameter("gpk", [2 * NF, 2 * GW], BF16,
                                    isOutput=False)
    # x^T in bf16 hi/lo split stacked along K (rows 0:64 hi, 64:128 lo); one
    # K=128 matmul per gather reconstructs exact-f32 x values in PSUM.
    XW = BL
    xt2 = nc.declare_dram_parameter("xt2", [2 * NF, XW], BF16, isOutput=False)
    # f32 pack:
    #   cols 0:BL       rows 0:65  = x^T plus ones row
    #   col  BL         rows 0:65  = [w_lin; b_fc+b_lin]
    #   cols BL+1:BL+65 rows 0:126 = scal[q, t*4+c] (cneg, cpos-cneg, u, one)
    PK = BL + 1 + NT * 4
    pack = nc.declare_dram_parameter("pack", [128, PK], F32, isOutput=False)
    # u / ones reduction weights in the compute dtype (rhs of the reduction
    # matmuls must match the E/G stationary dtype)
    packh = nc.declare_dram_parameter("packh", [TP, NT, 2], CD, isOutput=False)
    out = nc.declare_dram_parameter("out", [128, 2], F32, isOutput=True)
    if debug:
        dbg_s = nc.declare_dram_parameter("dbg_s", [NT, TP, BL], F32,
                                          isOutput=True)
        dbg_e = nc.declare_dram_parameter("dbg_e", [NT, TP, BL], F32,
                                          isOutput=True)
        dbg_nz = nc.declare_dram_parameter("dbg_nz", [128, 4], F32,
                                           isOutput=True)
        dbg_lin = nc.declare_dram_parameter("dbg_lin", [128, 2], F32,
                                            isOutput=True)

    with TileContext(nc, pool_alloc_mode=pool_mode) as tc, \
            ExitStack() as ctx:
        singles = ctx.enter_context(tc.tile_pool(name="singles", bufs=1))
        work = ctx.enter_context(tc.tile_pool(name="work", bufs=bufs_work))
        # DVE-written, PE-read tiles must not share slots (a reuse would add
        # a PE-release wait on a DVE instruction that already waits on ACT)
        gwork = ctx.enter_context(tc.tile_pool(name="gwork", bufs=NT))
        pmm = ctx.enter_context(tc.tile_pool(name="pmm", bufs=bufs_pmm,
                                             space="PSUM"))
        pacc = ctx.enter_context(tc.tile_pool(name="pacc", bufs=1, space="PSUM"))

        # DMA issue order matters: HWDGE issues serialize at ~625ns each, so
        # the tensors the first gather matmul needs (x^T, first gather chunk)
        # go first; the f32/fp16 packs are not needed until ~5us in.
        sb_x2a = singles.tile([2 * NF, XW], BF16)
        sb_x2 = sb_x2a[:, 0:BL]
        if swdge_x:
            nc.gpsimd.dma_start(out=sb_x2a[:, :], in_=xt2[:, :])
        else:
            nc.scalar.dma_start(out=sb_x2a[:, :], in_=xt2[:, :])
        sb_gpk = singles.tile([2 * NF, 2 * GW], BF16)
        # graduated chunks (tile edges) so early gathers start early
        tedges = [0, 2, 6, 10, 16] if chunks is None else chunks
        nc.sync.dma_start(
            out=sb_gpk[:, 2 * tedges[0] * TP:2 * tedges[1] * TP],
            in_=gpk[:, 2 * tedges[0] * TP:2 * tedges[1] * TP])
        sb_pack = singles.tile([128, PK], F32)
        nc.scalar.dma_start(out=sb_pack[:, :], in_=pack[:, :])
        sb_ch = singles.tile([TP, NT, 2], CD)
        nc.scalar.dma_start(out=sb_ch[:, :, :], in_=packh[:, :, :])
        for q in range(1, len(tedges) - 1):
            nc.sync.dma_start(
                out=sb_gpk[:, 2 * tedges[q] * TP:2 * tedges[q + 1] * TP],
                in_=gpk[:, 2 * tedges[q] * TP:2 * tedges[q + 1] * TP])

        def g_cols(t):
            return sb_gpk[:, 2 * t * TP:2 * (t + 1) * TP]
        sb_x = sb_pack[0:NF + 1, 0:BL]
        sb_w = sb_pack[0:NF + 1, BL:BL + 1]
        sb_c = sb_pack[0:TP, BL + 1:PK].rearrange("p (t c) -> p t c", c=4)

        # Per-tile partial reductions: [128, t, c] c: 0,1 = num halves;
        # 2,3 = Z halves.  Each column written exactly once (no PSUM
        # accumulation: start=True zero-marks the whole bank, so interleaved
        # accumulation chains in one bank corrupt each other).
        p_nz = pacc.tile([128, NT, 4], F32)
        p_lin = pacc.tile([128, 2], F32)

        if fp16:
            neg4 = singles.tile([TP, 1], F32)
            nc.vector.memset(neg4[:, :], -4.0)

        if warm:
            # dummy matmuls fill the PE's idle window while the input DMAs
            # stream, so the HAM clock gate is already released (2.4 GHz)
            # when the first real gather arrives (~3.4us of sustained PE
            # activity unthrottles the array)
            wsrc = singles.tile([128, BL], BF16)
            nc.gpsimd.memset(wsrc[:, :], 0.0)
            pwrm = pacc.tile([128, BL], F32)
            for _ in range(warm):
                nc.tensor.matmul(pwrm[:, :], lhsT=wsrc[:, 0:128],
                                 rhs=wsrc[:, :], start=True, stop=True,
                                 skip_group_check=True)

        # variable supertile sizes: small groups at the start (pipeline fills
        # sooner after the first DMA chunk) and at the end (shorter serial
        # drain chain into the final reduction)
        if groups is None:
            if st == 2:
                groups = [1, 1] + [2] * 5 + [1] * 4
            else:
                groups = [st] * (NT // st)
        assert sum(groups) == NT

        tbase = 0
        for s, stg in enumerate(groups):
            ptiles = [tbase + k for k in range(stg)]
            tbase += stg
            # Xi^T, Xj^T gathers: [TP, BL] = Gi2_t^T @ [x_hi; x_lo]^T (exact).
            # st subtiles side by side in one PSUM bank — each column range is
            # written exactly once, so the whole-bank zero-mark of a later
            # start=True does not corrupt earlier data.
            pxi = pmm.tile([TP, stg, BL], F32, tag="pxi")
            pxj = pmm.tile([TP, stg, BL], F32, tag="pxj")
            for k, t in enumerate(ptiles):
                gsl = g_cols(t)
                nc.tensor.matmul(
                    pxi[:, k, :], lhsT=gsl[:, 0:TP],
                    rhs=sb_x2[:, :], start=True, stop=True,
                )
                nc.tensor.matmul(
                    pxj[:, k, :], lhsT=gsl[:, TP:2 * TP],
                    rhs=sb_x2[:, :], start=True, stop=True,
                )

            # Stage Xi through ScalarE (DVE may read at most one PSUM
            # operand).  On alternate supertiles stage Xj through ScalarE
            # too: the product then runs in the DVE fp16 2x mode, balancing
            # the PSUM-evacuation cost between the two engines.
            # on direct groups the S product runs 1c/elem regardless (PSUM
            # operand), so keep xi in f32 there — S gets a single fp16
            # rounding instead of two
            # score entirely on ScalarE:  sc = sgn*prelu_alpha(c*S) with
            # E = exp(sgn*sc' - 4) folding the sign via the exp scale.
            # VectorE evacuates Xi (f32, so S gets a single fp16 rounding)
            # and computes S and G.
            xi_sb = work.tile([TP, stg, BL], F32, tag="xi")
            nc.vector.tensor_copy(out=xi_sb[:, :, :], in_=pxi[:, :, :])
            s_t = work.tile([TP, stg, BL], CD, tag="s")
            nc.vector.tensor_tensor(out=s_t[:, :, :], in0=xi_sb[:, :, :],
                                    in1=pxj[:, :, :], op=OP.mult)
            sc = work.tile([TP, stg, BL], F32, tag="sc")
            e_t = work.tile([TP, stg, BL], CD, tag="e")
            if s >= len(groups) - dve_score:
                # tail groups: score on VectorE (ACT is the serial driver
                # in the drain); those tiles' scal cols hold (cpmn, cneg)
                a1 = work.tile([TP, stg, BL], CD, tag="a1")
                for k, t in enumerate(ptiles):
                    nc.vector.tensor_scalar(
                        out=a1[:, k, :], in0=s_t[:, k, :], scalar1=0.0,
                        scalar2=sb_c[:, t, 0:1], op0=OP.max, op1=OP.mult)
                for k, t in enumerate(ptiles):
                    nc.vector.scalar_tensor_tensor(
                        out=sc[:, k, :], in0=s_t[:, k, :],
                        scalar=sb_c[:, t, 1:2], in1=a1[:, k, :],
                        op0=OP.mult, op1=OP.add)
                nc.scalar.activation(out=e_t[:, :, :], in_=sc[:, :, :],
                                     func=AF.Exp, bias=neg4[:, 0:1])
            else:
                for k, t in enumerate(ptiles):
                    nc.scalar.activation(
                        out=sc[:, k, :], in_=s_t[:, k, :], func=AF.Prelu,
                        scale=sb_c[:, t, 0:1], alpha=sb_c[:, t, 1:2])
                for k, t in enumerate(ptiles):
                    nc.scalar.activation(
                        out=e_t[:, k, :], in_=sc[:, k, :], func=AF.Exp,
                        scale=sb_c[:, t, 2:3], bias=neg4[:, 0:1])
            g_t = gwork.tile([TP, stg, BL], CD, tag="g")
            nc.vector.tensor_tensor(out=g_t[:, :, :], in0=e_t[:, :, :],
                                    in1=s_t[:, :, :], op=OP.mult)
            if debug:
                for k, t in enumerate(ptiles):
                    nc.sync.dma_start(out=dbg_s[t, :, :], in_=s_t[:, k, :])
                    nc.sync.dma_start(out=dbg_e[t, :, :], in_=e_t[:, k, :])

            for k, t in enumerate(ptiles):
                for h in range(2):
                    bsl = slice(h * 128, (h + 1) * 128)
                    nc.tensor.matmul(
                        p_nz[:, t, h:h + 1], lhsT=g_t[:, k, bsl],
                        rhs=sb_ch[:, t, 0:1],
                        start=True, stop=True, skip_group_check=True,
                    )
                    nc.tensor.matmul(
                        p_nz[:, t, 2 + h:3 + h], lhsT=e_t[:, k, bsl],
                        rhs=sb_ch[:, t, 1:2],
                        start=True, stop=True, skip_group_check=True,
                    )

        # x @ w_lin (+ bias via the ones row): one matmul per b-half
        for h in range(2):
            nc.tensor.matmul(
                p_lin[:, h:h + 1],
                lhsT=sb_x[:, h * 128:(h + 1) * 128],
                rhs=sb_w[:, :],
                start=True, stop=True,
            )

        # sum the 16 per-tile partials: [128, (c,t)] -> [128, 4]
        nz = work.tile([128, 4], F32, tag="fin4")
        nc.vector.reduce_sum(
            out=nz[:, :], in_=p_nz[:, :, :].rearrange("p t c -> p c t"),
            axis=mybir.AxisListType.X,
        )

        if debug:
            nc.sync.dma_start(out=dbg_nz[:, :], in_=nz[:, :])
            clin = work.tile([128, 2], F32, tag="dbgc2")
            nc.scalar.activation(out=clin[:, :], in_=p_lin[:, :], func=AF.Copy)
            nc.sync.dma_start(out=dbg_lin[:, :], in_=clin[:, :])

        # logits = num/Z + xlin ; out = 1/(1+exp(-logits))
        rz = work.tile([128, 2], F32, tag="fin")
        nc.vector.reciprocal(rz[:, :], nz[:, 2:4])
        # logit col h = num_h * (1/Z_h) + lin_h, fused per column
        lg = work.tile([128, 2], F32, tag="fin")
        for h in range(2):
            nc.vector.scalar_tensor_tensor(
                out=lg[:, h:h + 1], in0=nz[:, h:h + 1],
                scalar=rz[:, h:h + 1], in1=p_lin[:, h:h + 1],
                op0=OP.mult, op1=OP.add)
        # sigmoid(x) = (1 + tanh(x/2)) / 2 — tanh shares the ACT table set
        # with exp (no table switch), and the affine is one dual-op TS
        th = work.tile([128, 2], F32, tag="fin")
        nc.scalar.activation(out=th[:, :], in_=lg[:, :], func=AF.Tanh,
                             scale=0.5)
        o = work.tile([128, 2], F32, tag="fin")
        nc.vector.tensor_scalar(out=o[:, :], in0=th[:, :], scalar1=1.0,
                                scalar2=0.5, op0=OP.add, op1=OP.mult)
        if swdge_out:
            nc.gpsimd.dma_start(out=out[:, :], in_=o[:, :])
        else:
            nc.sync.dma_start(out=out[:, :], in_=o[:, :])

    if compile:
        nc.compile()
    return nc


def _get_nc():
    if "nc" not in _BUILT:
        _BUILT["nc"] = _build_nc(bufs_work=12, dve_score=DVE_SCORE)
    return _BUILT["nc"]


def _host_prep(x, V, w_att1, b_att1, w_att2, b_att2, w_fc, b_fc, w_lin, b_lin):
    """Fold the tiny replicated parameters into per-pair vectors (float64)."""
    ii, jj = np.triu_indices(NF, k=1)
    V64 = V.astype(np.float64)
    W = V64[ii] * V64[jj]                                  # [P, d]
    A = W @ w_att1.astype(np.float64).T                    # [P, att]
    w2 = w_att2.astype(np.float64)
    cpos = np.maximum(A, 0.0) @ w2
    cneg = np.minimum(A, 0.0) @ w2
    avec = 0.5 * (cpos + cneg)
    dvec = 0.5 * (cpos - cneg)
    u = W @ w_fc.astype(np.float64)
    const = float(b_fc) + float(b_lin)

    import ml_dtypes

    pidx = np.arange(P)
    t_, m_ = pidx // TP, pidx % TP

    # bf16 one-hot gathers, [Gi_t | Gj_t] interleaved per tile, rows
    # duplicated for the hi/lo x split
    GW = NT * TP
    gpk = np.zeros((2 * NF, 2 * GW), ml_dtypes.bfloat16)
    gpk[ii, 2 * t_ * TP + m_] = 1.0
    gpk[NF + ii, 2 * t_ * TP + m_] = 1.0
    gpk[jj, (2 * t_ + 1) * TP + m_] = 1.0
    gpk[NF + jj, (2 * t_ + 1) * TP + m_] = 1.0

    # score = cpos*S (S>=0) | cneg*S (S<0)  ==  sgn * lrelu_alpha(c*S)
    # with c = |cpos| (guarded), sgn = sign(cpos), alpha = cneg/(sgn*c)
    sgn = np.where(cpos >= 0, 1.0, -1.0)
    c_safe = np.maximum(np.abs(cpos), 1e-20)
    alpha = cneg / (sgn * c_safe)
    scal = np.zeros((TP, NT, 4), np.float32)
    scal[m_, t_, 0] = c_safe
    scal[m_, t_, 1] = alpha
    scal[m_, t_, 2] = sgn
    scal[m_, t_, 3] = cneg                # legacy path (lrelu=False)
    # trailing tiles use the VectorE score path: cols hold (cpmn, cneg)
    sel = t_ >= NT - DVE_SCORE
    scal[m_[sel], t_[sel], 0] = (cpos - cneg)[sel]
    scal[m_[sel], t_[sel], 1] = cneg[sel]
    scal2 = np.zeros((TP, NT, 2), np.float32)
    scal2[m_, t_, 0] = cpos - cneg        # legacy path
    scal2[m_, t_, 1] = 1.0

    wla = np.concatenate([w_lin.astype(np.float64), [const]]).astype(
        np.float32).reshape(NF + 1, 1)

    # packed f32 blob minus the per-core xTa block (filled in kernel())
    PK = BL + 1 + NT * 4
    pack = np.zeros((128, PK), np.float32)
    pack[0:NF + 1, BL:BL + 1] = wla
    pack[0:TP, BL + 1:PK] = scal.reshape(TP, NT * 4)
    # u / ones reduction weights in fp16 for the fp16 pipeline
    packh = np.zeros((TP, NT, 2), np.float16)
    packh[m_, t_, 0] = u
    packh[m_, t_, 1] = 1.0
    return gpk, pack, packh


def kernel(**inputs):
    from concourse.bass_utils import run_bass_kernel_spmd

    x = np.ascontiguousarray(np.asarray(inputs["x"], dtype=np.float32))
    assert x.shape == (B, NF), x.shape
    b_att1 = np.asarray(inputs["b_att1"], dtype=np.float64)
    assert np.allclose(b_att1, 0.0), "kernel specialization requires b_att1 == 0"

    gpk0, pack0, packh0 = _host_prep(
        x, np.asarray(inputs["V"]), np.asarray(inputs["w_att1"]), b_att1,
        np.asarray(inputs["w_att2"]), np.asarray(inputs["b_att2"]),
        np.asarray(inputs["w_fc"]), np.asarray(inputs["b_fc"]),
        np.asarray(inputs["w_lin"]), np.asarray(inputs["b_lin"]),
    )

    import ml_dtypes

    x_hi32 = x.astype(ml_dtypes.bfloat16).astype(np.float32)
    x_lo = (x - x_hi32).astype(ml_dtypes.bfloat16)
    x_hi = x_hi32.astype(ml_dtypes.bfloat16)

    in_maps = []
    for c in range(NCORES):
        sl = slice(c * BL, (c + 1) * BL)
        pack = pack0.copy()
        pack[0:NF, 0:BL] = x[sl].T
        pack[NF, 0:BL] = 1.0
        xt2 = np.concatenate([x_hi[sl].T, x_lo[sl].T], axis=0)  # [128, BL]
        in_maps.append({"gpk": gpk0, "xt2": np.ascontiguousarray(xt2),
                        "pack": pack, "packh": packh0})

    nc = _get_nc()
    results = run_bass_kernel_spmd(nc, in_maps, core_ids=list(range(NCORES)))

    outs = []
    for c in range(NCORES):
        res = results.results[c]["out"]                   # [128, 2]
        outs.append(res.T.reshape(-1))                    # b_local = h*128 + q
    return np.concatenate(outs).astype(np.float32)


# revision 110
# speedup vs baseline: 1.0127x; 1.0050x over previous
"""AFM (Attentional Factorization Machine) Trainium2 kernel, 8-core data parallel.

Reference computation (B=2048, n=64 features, d=64 emb, att=64):
    e[b,i,:]  = x[b,i] * V[i,:]
    prod      = e[:,ii,:] * e[:,jj,:]            (P = 2016 feature pairs)
    h         = relu(prod @ w_att1.T + b_att1)
    score     = h @ w_att2 + b_att2
    att       = softmax(score, axis=pairs)
    pooled    = sum_p att * prod
    out       = sigmoid(pooled @ w_fc + b_fc + x @ w_lin + b_lin)

Algebraic collapse used here (valid because b_att1 == 0 in this problem):
    prod[b,p,:] = S[b,p] * W[p,:]      where S = x[:,ii]*x[:,jj], W = V[ii]*V[jj]
    score[b,p]  = S*cpos[p] if S>=0 else S*cneg[p]
                = S*a[p] + |S|*d[p]
      with A = W @ w_att1.T, cpos = relu(A)@w_att2, cneg = min(A,0)@w_att2,
           a = (cpos+cneg)/2, d = (cpos-cneg)/2
    pooled @ w_fc = (sum_p att*S*u) with u = W @ w_fc
    => out = sigmoid( (sum_p E*S*u)/(sum_p E) + x@w_lin + b_fc + b_lin ),
       E = exp(score)   (scores are O(10), no max-subtraction needed)

The two-sided score is a single parametric relu:
    score = sgn * prelu_alpha(c*S)   with c = |cpos| (guarded), sgn =
    sign(cpos), alpha = cneg/cpos — verified on-silicon that Prelu honours
    per-partition scale/alpha operands (Lrelu ignores alpha, fixed 0.01).

Device layout: pairs p on partitions (16 tiles of 126), local batch (256) on
the free axis.  S is produced by exact bf16 hi/lo one-hot gather matmuls
(Gi/Gj); VectorE evacuates Xi from PSUM (f32) and forms S = Xi*Xj (fp16) and
G = E*S; ScalarE computes the whole score path: sc = prelu(c*S, alpha) in
f32 and E = exp(sgn*sc - 4) in fp16 (the -4 shift keeps E and G in fp16
range; the num/Z softmax ratio is shift-invariant).  Both softmax reductions
run as PE matmuls with the E/G tiles as the stationary operand (moving
operand = the u / ones column).  Measured output max rel err vs the
reference: 7.26e-3 (gate: 2e-2); per-core exec ~23.6us (CoreSim timeline).
"""

import numpy as np

B = 2048
NF = 64          # features
NCORES = 8
BL = B // NCORES  # 256 local batch
NT = 16          # pair tiles
TP = 126         # pairs per tile (partitions)
P = NT * TP      # 2016

_BUILT = {}
DVE_SCORE = 2   # trailing singleton groups compute score on VectorE


def _build_nc(debug=False, compile=True, bufs_work=3, bufs_pmm=2,
              xj_direct=True, a1_act=False, st=2, fp16=True,
              groups=None, chunks=None, direct_mod=2, direct_set=None,
              lrelu=True, warm=0, copy_act_set=(), swdge_x=False,
              swdge_out=False, pool_mode="stack", dve_score=0):
    import concourse.mybir as mybir
    from concourse.bacc import Bacc
    from concourse.tile import TileContext
    from contextlib import ExitStack

    F32 = mybir.dt.float32
    BF16 = mybir.dt.bfloat16
    F16 = mybir.dt.float16
    # elementwise compute dtype: fp16 (10-bit mantissa) keeps the final
    # rel err ~1.4e-2 (< the 2e-2 gate, deterministic inputs) and unlocks
    # the DVE 2x/4x modes; exp is shifted by -4 so E=exp(score-4) and
    # G=E*S stay in fp16 range (the num/Z softmax ratio is shift-invariant)
    CD = F16 if fp16 else F32
    AF = mybir.ActivationFunctionType
    OP = mybir.AluOpType

    nc = Bacc()  # Bacc.compile() legalizes multi-wait instructions (the TPB
    # ISA allows one sync wait per instruction) into EventSemaphores
    #
    # bf16 one-hot gathers [128, NT*2*TP]: cols t*252:(t+1)*252 = [Gi_t|Gj_t],
    # rows duplicated (k and 64+k) for the hi/lo x split
    GW = NT * TP
    gpk = nc.declare_dram_parameter("gpk", [2 * NF, 2 * GW], BF16,
                                    isOutput=False)
    # x^T in bf16 hi/lo split stacked along K (rows 0:64 hi, 64:128 lo); one
    # K=128 matmul per gather reconstructs exact-f32 x values in PSUM.
    XW = BL
    xt2 = nc.declare_dram_parameter("xt2", [2 * NF, XW], BF16, isOutput=False)
    # f32 pack:
    #   cols 0:BL       rows 0:65  = x^T plus ones row
    #   col  BL         rows 0:65  = [w_lin; b_fc+b_lin]
    #   cols BL+1:BL+65 rows 0:126 = scal[q, t*4+c] (cneg, cpos-cneg, u, one)
    PK = BL + 1 + NT * 4
    pack = nc.declare_dram_parameter("pack", [128, PK], F32, isOutput=False)
    # u / ones reduction weights in the compute dtype (rhs of the reduction
    # matmuls must match the E/G stationary dtype)
    packh = nc.declare_dram_parameter("packh", [TP, NT, 2], CD, isOutput=False)
    out = nc.declare_dram_parameter("out", [128, 2], F32, isOutput=True)
    if debug:
        dbg_s = nc.declare_dram_parameter("dbg_s", [NT, TP, BL], F32,
                                          isOutput=True)
        dbg_e = nc.declare_dram_parameter("dbg_e", [NT, TP, BL], F32,
                                          isOutput=True)
        dbg_nz = nc.declare_dram_parameter("dbg_nz", [128, 4], F32,
                                           isOutput=True)
        dbg_lin = nc.declare_dram_parameter("dbg_lin", [128, 2], F32,
                                            isOutput=True)

    with TileContext(nc, pool_alloc_mode=pool_mode) as tc, \
            ExitStack() as ctx:
        singles = ctx.enter_context(tc.tile_pool(name="singles", bufs=1))
        work = ctx.enter_context(tc.tile_pool(name="work", bufs=bufs_work))
        # DVE-written, PE-read tiles must not share slots (a reuse would add
        # a PE-release wait on a DVE instruction that already waits on ACT)
        gwork = ctx.enter_context(tc.tile_pool(name="gwork", bufs=NT))
        pmm = ctx.enter_context(tc.tile_pool(name="pmm", bufs=bufs_pmm,
                                             space="PSUM"))
        pacc = ctx.enter_context(tc.tile_pool(name="pacc", bufs=1, space="PSUM"))

        # DMA issue order matters: HWDGE issues serialize at ~625ns each, so
        # the tensors the first gather matmul needs (x^T, first gather chunk)
        # go first; the f32/fp16 packs are not needed until ~5us in.
        sb_x2a = singles.tile([2 * NF, XW], BF16)
        sb_x2 = sb_x2a[:, 0:BL]
        if swdge_x:
            nc.gpsimd.dma_start(out=sb_x2a[:, :], in_=xt2[:, :])
        else:
            nc.scalar.dma_start(out=sb_x2a[:, :], in_=xt2[:, :])
        sb_gpk = singles.tile([2 * NF, 2 * GW], BF16)
        # graduated chunks (tile edges) so early gathers start early
        tedges = [0, 2, 6, 10, 16] if chunks is None else chunks
        nc.sync.dma_start(
            out=sb_gpk[:, 2 * tedges[0] * TP:2 * tedges[1] * TP],
            in_=gpk[:, 2 * tedges[0] * TP:2 * tedges[1] * TP])
        sb_pack = singles.tile([128, PK], F32)
        nc.scalar.dma_start(out=sb_pack[:, :], in_=pack[:, :])
        sb_ch = singles.tile([TP, NT, 2], CD)
        nc.scalar.dma_start(out=sb_ch[:, :, :], in_=packh[:, :, :])
        for q in range(1, len(tedges) - 1):
            nc.sync.dma_start(
                out=sb_gpk[:, 2 * tedges[q] * TP:2 * tedges[q + 1] * TP],
                in_=gpk[:, 2 * tedges[q] * TP:2 * tedges[q + 1] * TP])

        def g_cols(t):
            return sb_gpk[:, 2 * t * TP:2 * (t + 1) * TP]
        sb_x = sb_pack[0:NF + 1, 0:BL]
        sb_w = sb_pack[0:NF + 1, BL:BL + 1]
        sb_c = sb_pack[0:TP, BL + 1:PK].rearrange("p (t c) -> p t c", c=4)

        # Per-tile partial reductions: [128, t, c] c: 0,1 = num halves;
        # 2,3 = Z halves.  Each column written exactly once (no PSUM
        # accumulation: start=True zero-marks the whole bank, so interleaved
        # accumulation chains in one bank corrupt each other).
        p_nz = pacc.tile([128, NT, 4], F32)
        p_lin = pacc.tile([128, 2], F32)

        if fp16:
            neg4 = singles.tile([TP, 1], F32)
            nc.vector.memset(neg4[:, :], -4.0)

        if warm:
            # dummy matmuls fill the PE's idle window while the input DMAs
            # stream, so the HAM clock gate is already released (2.4 GHz)
            # when the first real gather arrives (~3.4us of sustained PE
            # activity unthrottles the array)
            wsrc = singles.tile([128, BL], BF16)
            nc.gpsimd.memset(wsrc[:, :], 0.0)
            pwrm = pacc.tile([128, BL], F32)
            for _ in range(warm):
                nc.tensor.matmul(pwrm[:, :], lhsT=wsrc[:, 0:128],
                                 rhs=wsrc[:, :], start=True, stop=True,
                                 skip_group_check=True)

        # variable supertile sizes: small groups at the start (pipeline fills
        # sooner after the first DMA chunk) and at the end (shorter serial
        # drain chain into the final reduction)
        if groups is None:
            if st == 2:
                groups = [1, 1] + [2] * 5 + [1] * 4
            else:
                groups = [st] * (NT // st)
        assert sum(groups) == NT

        tbase = 0
        for s, stg in enumerate(groups):
            ptiles = [tbase + k for k in range(stg)]
            tbase += stg
            # Xi^T, Xj^T gathers: [TP, BL] = Gi2_t^T @ [x_hi; x_lo]^T (exact).
            # st subtiles side by side in one PSUM bank — each column range is
            # written exactly once, so the whole-bank zero-mark of a later
            # start=True does not corrupt earlier data.
            pxi = pmm.tile([TP, stg, BL], F32, tag="pxi")
            pxj = pmm.tile([TP, stg, BL], F32, tag="pxj")
            for k, t in enumerate(ptiles):
                gsl = g_cols(t)
                nc.tensor.matmul(
                    pxi[:, k, :], lhsT=gsl[:, 0:TP],
                    rhs=sb_x2[:, :], start=True, stop=True,
                )
                nc.tensor.matmul(
                    pxj[:, k, :], lhsT=gsl[:, TP:2 * TP],
                    rhs=sb_x2[:, :], start=True, stop=True,
                )

            # Stage Xi through ScalarE (DVE may read at most one PSUM
            # operand).  On alternate supertiles stage Xj through ScalarE
            # too: the product then runs in the DVE fp16 2x mode, balancing
            # the PSUM-evacuation cost between the two engines.
            # on direct groups the S product runs 1c/elem regardless (PSUM
            # operand), so keep xi in f32 there — S gets a single fp16
            # rounding instead of two
            # score entirely on ScalarE:  sc = sgn*prelu_alpha(c*S) with
            # E = exp(sgn*sc' - 4) folding the sign via the exp scale.
            # VectorE evacuates Xi (f32, so S gets a single fp16 rounding)
            # and computes S and G.
            xi_sb = work.tile([TP, stg, BL], F32, tag="xi")
            nc.vector.tensor_copy(out=xi_sb[:, :, :], in_=pxi[:, :, :])
            s_t = work.tile([TP, stg, BL], CD, tag="s")
            nc.vector.tensor_tensor(out=s_t[:, :, :], in0=xi_sb[:, :, :],
                                    in1=pxj[:, :, :], op=OP.mult)
            sc = work.tile([TP, stg, BL], F32, tag="sc")
            e_t = work.tile([TP, stg, BL], CD, tag="e")
            if s >= len(groups) - dve_score:
                # tail groups: score on VectorE (ACT is the serial driver
                # in the drain); those tiles' scal cols hold (cpmn, cneg)
                a1 = work.tile([TP, stg, BL], CD, tag="a1")
                for k, t in enumerate(ptiles):
                    nc.vector.tensor_scalar(
                        out=a1[:, k, :], in0=s_t[:, k, :], scalar1=0.0,
                        scalar2=sb_c[:, t, 0:1], op0=OP.max, op1=OP.mult)
                for k, t in enumerate(ptiles):
                    nc.vector.scalar_tensor_tensor(
                        out=sc[:, k, :], in0=s_t[:, k, :],
                        scalar=sb_c[:, t, 1:2], in1=a1[:, k, :],
                        op0=OP.mult, op1=OP.add)
                nc.scalar.activation(out=e_t[:, :, :], in_=sc[:, :, :],
                                     func=AF.Exp, bias=neg4[:, 0:1])
            else:
                for k, t in enumerate(ptiles):
                    nc.scalar.activation(
                        out=sc[:, k, :], in_=s_t[:, k, :], func=AF.Prelu,
                        scale=sb_c[:, t, 0:1], alpha=sb_c[:, t, 1:2])
                for k, t in enumerate(ptiles):
                    nc.scalar.activation(
                        out=e_t[:, k, :], in_=sc[:, k, :], func=AF.Exp,
                        scale=sb_c[:, t, 2:3], bias=neg4[:, 0:1])
            g_t = gwork.tile([TP, stg, BL], CD, tag="g")
            nc.vector.tensor_tensor(out=g_t[:, :, :], in0=e_t[:, :, :],
                                    in1=s_t[:, :, :], op=OP.mult)
            if debug:
                for k, t in enumerate(ptiles):
                    nc.sync.dma_start(out=dbg_s[t, :, :], in_=s_t[:, k, :])
                    nc.sync.dma_start(out=dbg_e[t, :, :], in_=e_t[:, k, :])

            for k, t in enumerate(ptiles):
                for h in range(2):
                    bsl = slice(h * 128, (h + 1) * 128)
                    nc.tensor.matmul(
                        p_nz[:, t, h:h + 1], lhsT=g_t[:, k, bsl],
                        rhs=sb_ch[:, t, 0:1],
                        start=True, stop=True, skip_group_check=True,
                    )
                    nc.tensor.matmul(
                        p_nz[:, t, 2 + h:3 + h], lhsT=e_t[:, k, bsl],
                        rhs=sb_ch[:, t, 1:2],
                        start=True, stop=True, skip_group_check=True,
                    )

        # x @ w_lin (+ bias via the ones row): one matmul per b-half
        for h in range(2):
            nc.tensor.matmul(
                p_lin[:, h:h + 1],
                lhsT=sb_x[:, h * 128:(h + 1) * 128],
                rhs=sb_w[:, :],
                start=True, stop=True,
            )

        # sum the 16 per-tile partials: [128, (c,t)] -> [128, 4]
        nz = work.tile([128, 4], F32, tag="fin4")
        nc.vector.reduce_sum(
            out=nz[:, :], in_=p_nz[:, :, :].rearrange("p t c -> p c t"),
            axis=mybir.AxisListType.X,
        )

        if debug:
            nc.sync.dma_start(out=dbg_nz[:, :], in_=nz[:, :])
            clin = work.tile([128, 2], F32, tag="dbgc2")
            nc.scalar.activation(out=clin[:, :], in_=p_lin[:, :], func=AF.Copy)
            nc.sync.dma_start(out=dbg_lin[:, :], in_=clin[:, :])

        # logits = num/Z + xlin ; out = 1/(1+exp(-logits))
        rz = work.tile([128, 2], F32, tag="fin")
        nc.vector.reciprocal(rz[:, :], nz[:, 2:4])
        # logit col h = num_h * (1/Z_h) + lin_h, fused per column
        lg = work.tile([128, 2], F32, tag="fin")
        for h in range(2):
            nc.vector.scalar_tensor_tensor(
                out=lg[:, h:h + 1], in0=nz[:, h:h + 1],
                scalar=rz[:, h:h + 1], in1=p_lin[:, h:h + 1],
                op0=OP.mult, op1=OP.add)
        # sigmoid(x) = (1 + tanh(x/2)) / 2 — tanh shares the ACT table set
        # with exp (no table switch), and the affine is one dual-op TS
        th = work.tile([128, 2], F32, tag="fin")
        nc.scalar.activation(out=th[:, :], in_=lg[:, :], func=AF.Tanh,
                             scale=0.5)
        o = work.tile([128, 2], F32, tag="fin")
        nc.vector.tensor_scalar(out=o[:, :], in0=th[:, :], scalar1=1.0,
                                scalar2=0.5, op0=OP.add, op1=OP.mult)
        if swdge_out:
            nc.gpsimd.dma_start(out=out[:, :], in_=o[:, :])
        else:
            nc.sync.dma_start(out=out[:, :], in_=o[:, :])

    if compile:
        nc.compile()
    return nc


def _get_nc():
    if "nc" not in _BUILT:
        _BUILT["nc"] = _build_nc(bufs_work=12, dve_score=DVE_SCORE,
                                 groups=[1, 1] + [2] * 5 + [2, 1, 1])
    return _BUILT["nc"]


def _host_prep(x, V, w_att1, b_att1, w_att2, b_att2, w_fc, b_fc, w_lin, b_lin):
    """Fold the tiny replicated parameters into per-pair vectors (float64)."""
    ii, jj = np.triu_indices(NF, k=1)
    V64 = V.astype(np.float64)
    W = V64[ii] * V64[jj]                                  # [P, d]
    A = W @ w_att1.astype(np.float64).T                    # [P, att]
    w2 = w_att2.astype(np.float64)
    cpos = np.maximum(A, 0.0) @ w2
    cneg = np.minimum(A, 0.0) @ w2
    avec = 0.5 * (cpos + cneg)
    dvec = 0.5 * (cpos - cneg)
    u = W @ w_fc.astype(np.float64)
    const = float(b_fc) + float(b_lin)

    import ml_dtypes

    pidx = np.arange(P)
    t_, m_ = pidx // TP, pidx % TP

    # bf16 one-hot gathers, [Gi_t | Gj_t] interleaved per tile, rows
    # duplicated for the hi/lo x split
    GW = NT * TP
    gpk = np.zeros((2 * NF, 2 * GW), ml_dtypes.bfloat16)
    gpk[ii, 2 * t_ * TP + m_] = 1.0
    gpk[NF + ii, 2 * t_ * TP + m_] = 1.0
    gpk[jj, (2 * t_ + 1) * TP + m_] = 1.0
    gpk[NF + jj, (2 * t_ + 1) * TP + m_] = 1.0

    # score = cpos*S (S>=0) | cneg*S (S<0)  ==  sgn * lrelu_alpha(c*S)
    # with c = |cpos| (guarded), sgn = sign(cpos), alpha = cneg/(sgn*c)
    sgn = np.where(cpos >= 0, 1.0, -1.0)
    c_safe = np.maximum(np.abs(cpos), 1e-20)
    alpha = cneg / (sgn * c_safe)
    scal = np.zeros((TP, NT, 4), np.float32)
    scal[m_, t_, 0] = c_safe
    scal[m_, t_, 1] = alpha
    scal[m_, t_, 2] = sgn
    scal[m_, t_, 3] = cneg                # legacy path (lrelu=False)
    # trailing tiles use the VectorE score path: cols hold (cpmn, cneg)
    sel = t_ >= NT - DVE_SCORE
    scal[m_[sel], t_[sel], 0] = (cpos - cneg)[sel]
    scal[m_[sel], t_[sel], 1] = cneg[sel]
    scal2 = np.zeros((TP, NT, 2), np.float32)
    scal2[m_, t_, 0] = cpos - cneg        # legacy path
    scal2[m_, t_, 1] = 1.0

    wla = np.concatenate([w_lin.astype(np.float64), [const]]).astype(
        np.float32).reshape(NF + 1, 1)

    # packed f32 blob minus the per-core xTa block (filled in kernel())
    PK = BL + 1 + NT * 4
    pack = np.zeros((128, PK), np.float32)
    pack[0:NF + 1, BL:BL + 1] = wla
    pack[0:TP, BL + 1:PK] = scal.reshape(TP, NT * 4)
    # u / ones reduction weights in fp16 for the fp16 pipeline
    packh = np.zeros((TP, NT, 2), np.float16)
    packh[m_, t_, 0] = u
    packh[m_, t_, 1] = 1.0
    return gpk, pack, packh


def kernel(**inputs):
    from concourse.bass_utils import run_bass_kernel_spmd

    x = np.ascontiguousarray(np.asarray(inputs["x"], dtype=np.float32))
    assert x.shape == (B, NF), x.shape
    b_att1 = np.asarray(inputs["b_att1"], dtype=np.float64)
    assert np.allclose(b_att1, 0.0), "kernel specialization requires b_att1 == 0"

    gpk0, pack0, packh0 = _host_prep(
        x, np.asarray(inputs["V"]), np.asarray(inputs["w_att1"]), b_att1,
        np.asarray(inputs["w_att2"]), np.asarray(inputs["b_att2"]),
        np.asarray(inputs["w_fc"]), np.asarray(inputs["b_fc"]),
        np.asarray(inputs["w_lin"]), np.asarray(inputs["b_lin"]),
    )

    import ml_dtypes

    x_hi32 = x.astype(ml_dtypes.bfloat16).astype(np.float32)
    x_lo = (x - x_hi32).astype(ml_dtypes.bfloat16)
    x_hi = x_hi32.astype(ml_dtypes.bfloat16)

    in_maps = []
    for c in range(NCORES):
        sl = slice(c * BL, (c + 1) * BL)
        pack = pack0.copy()
        pack[0:NF, 0:BL] = x[sl].T
        pack[NF, 0:BL] = 1.0
        xt2 = np.concatenate([x_hi[sl].T, x_lo[sl].T], axis=0)  # [128, BL]
        in_maps.append({"gpk": gpk0, "xt2": np.ascontiguousarray(xt2),
                        "pack": pack, "packh": packh0})

    nc = _get_nc()
    results = run_bass_kernel_spmd(nc, in_maps, core_ids=list(range(NCORES)))

    outs = []
    for c in range(NCORES):
        res = results.results[c]["out"]                   # [128, 2]
        outs.append(res.T.reshape(-1))                    # b_local = h*128 + q
    return np.concatenate(outs).astype(np.float32)
